# revision 76
# baseline (speedup 1.0000x reference)
# kernel.py — Trainium2 Bass kernel for nn_DenseGridNet (bilinear grid sample + MLP)
#
# v2 strategy (data-parallel over 8 NeuronCores, sorted + PAIRED sharding):
#  * host: computes cell ids + bilinear coefs (exact fp32 replication of the
#    reference), sorts points by cell, PAIRS adjacent points whose cells are
#    <=1 apart (3% dummy padding), and uploads:
#      - per-chunk windows of a 256B-row patch table (row r = fp16 patches of
#        cells r..r+7 in [h][f][j] order; pairs read the first 2 patches),
#      - int16 pair indices (one 256B gather descriptor per PAIR -> half the
#        descriptors of per-point gathering),
#      - fp16 c8 coefs (4 bilinear coefs placed in the pair-half the point
#        occupies, zeros in the other half),
#      - fp16 idf rows pre-transposed into PSUM column order.
#  * device per 8192-point chunk: one dma_gather of 4096 256B rows; one
#    GpSimd 2x-mode fp16 multiply q = c8 (x) patch (GpSimd so DVE stays
#    purely on back-end relu work); one xbar DMA transpose to feature-major;
#    TensorE: K=2 idf matmul accumulated with the q32 matmuls (layer1), then
#    block-diagonal layer2/layer3 (2 pts/col) in 512-col sub-quarters with
#    skewed emission; relu via DVE/ACT split with bias add; sigmoid on ACT;
#    fp16 output.
import os
import numpy as np

RX = 1024
RY = 1024
F = 4
HID = 64
N_CORES = 8
P = 128          # partitions
SLOT = 32        # fp16 q slots per point ([h][f][j] of the 2-cell pair)
CHPAIRS = 4096   # gather descriptors (pairs) per chunk
CHPTS = 8192     # points per chunk (2 per pair, incl. dummies)
WIN = 6144       # window rows (256B each) per chunk


def _build_bass(n_chunks, front_swap=False):
    """Bass program for one core processing n_chunks*CHPTS (padded) points.

    front_swap flips the emission order of two independent front-end DMAs —
    semantically identical, but it perturbs the tile scheduler into a
    different (sometimes better) static schedule; kernel() compiles both
    when the first draw simulates poorly and keeps the faster one."""
    import concourse.bass as bass
    import concourse.tile as tile
    from concourse import bacc, library_config
    import concourse.mybir as mybir

    dt = mybir.dt
    NPAIR = n_chunks * CHPAIRS
    TCH = CHPTS // P              # 64 point cols per chunk
    GCH = CHPAIRS // P            # 32 pair cols per chunk
    QCOLS = 1024                  # psum cols per quarter (2048 points)

    nc = bacc.Bacc(None, target_bir_lowering=False)

    f32 = dt.float32
    f16 = dt.float16

    # ---- DRAM I/O -------------------------------------------------------
    ptw_d = nc.dram_tensor("ptw", [n_chunks * WIN, 128], f16,
                           kind="ExternalInput")
    idx_d = nc.dram_tensor("idx16", [P, n_chunks * (CHPAIRS // 16)], dt.int16,
                           kind="ExternalInput")
    co8_d = nc.dram_tensor("co8", [P, n_chunks * TCH, 8], f16,
                           kind="ExternalInput")
    idft_d = nc.dram_tensor("idft", [2, n_chunks * CHPAIRS], f16,
                            kind="ExternalInput")
    l1_d = nc.dram_tensor("lhsT1", [128, 128], f16, kind="ExternalInput")
    l12_d = nc.dram_tensor("lhsT12", [2, 128], f16, kind="ExternalInput")
    l2_d = nc.dram_tensor("lhsT2", [128, 128], f16, kind="ExternalInput")
    l3_d = nc.dram_tensor("lhsT3", [128, 8], f16, kind="ExternalInput")
    b1_d = nc.dram_tensor("b1rep", [128, 1], f32, kind="ExternalInput")
    b2_d = nc.dram_tensor("b2rep", [128, 1], f32, kind="ExternalInput")
    b3_d = nc.dram_tensor("b3rep", [8, 1], f32, kind="ExternalInput")
    out_d = nc.dram_tensor("yT", [6, NPAIR], f16, kind="ExternalOutput")

    with tile.TileContext(nc) as tc:
        with (
            tc.tile_pool(name="persist", bufs=1) as pp,
            tc.tile_pool(name="psum_l1", bufs=2, space="PSUM") as ps1,
            tc.tile_pool(name="psum_l2", bufs=2, space="PSUM") as ps2,
            tc.tile_pool(name="psum_l3", bufs=2, space="PSUM") as ps3,
            tc.tile_pool(name="mlp", bufs=4) as mp,
            tc.tile_pool(name="outp", bufs=3) as op_,
        ):
            l1w = pp.tile([128, 128], f16, tag="l1w")
            l12w = pp.tile([2, 128], f16, tag="l12w")
            l2w = pp.tile([128, 128], f16, tag="l2w")
            l3w = pp.tile([128, 8], f16, tag="l3w")
            b1r = pp.tile([128, 1], f32, tag="b1r")
            b2r = pp.tile([128, 1], f32, tag="b2r")
            b3r = pp.tile([8, 1], f32, tag="b3r")

            NBUF = 3
            NPB = 3
            pb = [pp.tile([P, GCH, 128], f16, name=f"pb{i}", tag=f"pb{i}")
                  for i in range(NPB)]
            qpm = [pp.tile([P, TCH, SLOT], f16, name=f"qpm{i}", tag=f"qpm{i}")
                   for i in range(NBUF)]
            NTB = 3
            qT = [pp.tile([P, TCH * SLOT // P, 128], f16, name=f"qT{i}",
                          tag=f"qT{i}") for i in range(NTB)]
            idft = [pp.tile([2, CHPAIRS], f16, name=f"idft{i}", tag=f"idft{i}")
                    for i in range(NBUF)]
            idxs = pp.tile([P, n_chunks * (CHPAIRS // 16)], dt.int16,
                           tag="idxs")
            co8s = pp.tile([P, n_chunks * TCH, 8], f16, tag="co8s")

            nc.sync.dma_start(idxs[:], idx_d[:])
            nc.sync.dma_start(l1w[:], l1_d[:])
            nc.sync.dma_start(l12w[:], l12_d[:])
            nc.sync.dma_start(l2w[:], l2_d[:])
            nc.sync.dma_start(l3w[:], l3_d[:])
            nc.sync.dma_start(b1r[:], b1_d[:])
            nc.sync.dma_start(b2r[:], b2_d[:])
            nc.sync.dma_start(b3r[:], b3_d[:])
            nc.sync.dma_start(co8s[:], co8_d[:])

            AL = mybir.AluOpType
            nc.gpsimd.load_library(library_config.mlp)

            def issue_front(chi):
                """idft load + patch gather for chunk chi."""
                i = chi % NBUF
                nc.sync.dma_start(
                    idft[i][:],
                    idft_d[:, chi * CHPAIRS:(chi + 1) * CHPAIRS])
                nc.gpsimd.dma_gather(
                    out_ap=pb[chi % NPB][:],
                    in_ap=ptw_d[chi * WIN:(chi + 1) * WIN, :],
                    idxs_ap=idxs[:, chi * (CHPAIRS // 16):
                                 (chi + 1) * (CHPAIRS // 16)],
                    num_idxs=CHPAIRS,
                    num_idxs_reg=CHPAIRS,
                    elem_size=128,
                    single_packet=False,
                )

            issue_front(0)
            for chi in range(n_chunks):
                i = chi % NBUF
                pbb = pb[chi % NPB]
                qb = qpm[i]
                tb = qT[chi % NTB]
                ib = idft[i]

                if chi + 1 < n_chunks:
                    issue_front(chi + 1)

                # q[p, g, d, s] = c8[p, 2g+d, s3(h,j)] * patch[p, g, s(h,f,j)]
                c0 = chi * TCH
                in1 = (co8s[:, c0:c0 + TCH, :]
                       .rearrange("p (g d) (h j) -> p g d h j", d=2, h=2)
                       .unsqueeze(4)
                       .to_broadcast([P, GCH, 2, 2, 4, 4]))
                in2 = (pbb[:, :, 0:SLOT]
                       .rearrange("p g (h f j) -> p g h f j", h=2, f=4)
                       .unsqueeze(2)
                       .to_broadcast([P, GCH, 2, 2, 4, 4]))
                outq = qb[:].rearrange("p (g d) (h f j) -> p g d h f j",
                                       d=2, h=2, f=4)
                # qmult lives on GpSimd: keeps DVE purely on back-end relu
                # work so front-end buffer waits can't starve the MLP.
                nc.gpsimd.tensor_tensor(outq, in1, in2, AL.mult)

                nc.sync.dma_start_transpose(
                    tb[:], qb[:].rearrange("p t s -> p (t s)"))

                y3c = op_.tile([6, CHPAIRS], f16, tag="y3c")
                H = QCOLS // 2       # 512: one psum bank
                l1pp = {}
                h1t = {}
                h2t = {}
                # skewed emission: every PE op's input is >=3 steps old,
                # hiding the relu handoff latency. relu1/sigmoid are merged
                # to 1024-wide ops (psum banks only constrain matmuls).
                for sq in range(14):
                    if sq < 8:
                        qq, half = sq // 2, sq % 2
                        rhs = tb[:, 4 * qq:4 * qq + 4, :]
                        c0 = sq * H
                        if half == 0:
                            l1pp[qq] = ps1.tile([P, QCOLS], f32, name="l1p",
                                                tag="l1p")
                        l1p = l1pp[qq]
                        hs = slice(half * H, half * H + H)
                        nc.tensor.matmul(l1p[:, hs], l12w[:], ib[:, c0:c0 + H],
                                         start=True, stop=False)
                        if half == 0:
                            nc.tensor.matmul(l1p[:, hs], l1w[0:64], rhs[0:64],
                                             start=False, stop=True)
                        else:
                            nc.tensor.matmul(l1p[:, hs], l1w[64:128],
                                             rhs[64:128],
                                             start=False, stop=True,
                                             tile_position=(64, 0))
                        if half == 1:
                            h1 = mp.tile([P, QCOLS], f16, tag="h1")
                            if qq < 3:
                                nc.vector.tensor_scalar(
                                    h1[:], l1p[:], b1r[:], 0.0, AL.add, AL.max)
                            else:
                                nc.scalar.activation(
                                    h1[:], l1p[:],
                                    mybir.ActivationFunctionType.Relu,
                                    bias=b1r[:])
                            h1t[qq] = h1
                    if 3 <= sq < 11:
                        s1 = sq - 3
                        h1 = h1t[s1 // 2]
                        l2p = ps2.tile([P, H], f32, tag="l2p")
                        nc.tensor.matmul(l2p[:],
                                         l2w[:],
                                         h1[:, (s1 % 2) * H:(s1 % 2) * H + H],
                                         start=True, stop=True)
                        if s1 % 2 == 1:
                            h1t.pop(s1 // 2)
                        h2 = mp.tile([P, H], f16, tag="h2")
                        if s1 in (0, 1, 2, 4, 5, 7):
                            nc.vector.tensor_scalar(h2[:], l2p[:],
                                                    b2r[:], 0.0, AL.add, AL.max)
                        else:
                            nc.scalar.activation(
                                h2[:], l2p[:],
                                mybir.ActivationFunctionType.Relu, bias=b2r[:])
                        h2t[s1] = h2
                    if sq >= 6:
                        s2 = sq - 6
                        l3p = ps3.tile([8, H], f32, tag="l3p")
                        nc.tensor.matmul(l3p[:], l3w[:], h2t.pop(s2)[:],
                                         start=True, stop=True)
                        nc.scalar.activation(y3c[:, s2 * H:(s2 + 1) * H],
                                             l3p[0:6, :],
                                             mybir.ActivationFunctionType.Sigmoid,
                                             bias=b3r[0:6])
                nc.scalar.dma_start(
                    out_d[:, chi * CHPAIRS:(chi + 1) * CHPAIRS], y3c[:])

    return nc


# ======================= host-side preparation ==========================

def _host_pointdata(x):
    """Exact fp32 replication of the reference's per-point index/weight math.

    Returns (cell[int64], c4[N,4] fp32 in corner order 00,10,01,11, idf)."""
    u = np.asarray(x[:, 1], np.float32)
    v = np.asarray(x[:, 2], np.float32)
    xu = u * np.float32(RX)
    yv = v * np.float32(RY)
    x0 = xu.astype(np.int32)
    x0 = np.where(x0 == RX, 0, x0)
    y0 = yv.astype(np.int32)
    wx = xu - x0.astype(np.float32)
    wy = yv - y0.astype(np.float32)
    cell = np.minimum(y0.astype(np.int64) * RX + x0, RX * RY - 1)
    one = np.float32(1.0)
    c00 = (one - wx) * (one - wy)
    c10 = wx * (one - wy)
    c01 = (one - wx) * wy
    c11 = wx * wy
    c4 = np.stack([c00, c10, c01, c11], axis=1)
    return cell, c4, np.asarray(x[:, 0], np.float32)


def _patch_table16(emb):
    """[RX*RY + 8, 16] fp16: per-cell patch in [f][j] order (j: 00,10,01,11),
    with x/y edge clamping baked in; 8 zero pad rows."""
    e = np.asarray(emb, dtype=np.float32).reshape(RY, RX, F)
    xs = np.arange(RX)
    x1 = np.minimum(xs + 1, RX - 1)
    ys = np.arange(RY)
    y1 = np.minimum(ys + 1, RY - 1)
    p = np.empty((RY, RX, F, 4), dtype=np.float32)   # [y, x, f, j]
    p[:, :, :, 0] = e
    p[:, :, :, 1] = e[:, x1, :]
    p[:, :, :, 2] = e[y1, :, :]
    p[:, :, :, 3] = e[y1][:, x1, :]
    pt = np.zeros((RY * RX + 8, 16), dtype=np.float16)
    pt[:RY * RX] = p.reshape(RY * RX, 16).astype(np.float16)
    return pt


def _pairing(cell_sorted):
    """Pair consecutive-cell points. Returns (pA, pB, base) index arrays into
    the SORTED order; pB == -1 for half-dummy pairs."""
    N = len(cell_sorted)
    counts = np.bincount(cell_sorted, minlength=RX * RY)
    starts = np.zeros(RX * RY + 1, np.int64)
    np.cumsum(counts, out=starts[1:])
    wi = np.arange(N) - starts[cell_sorted]          # within-cell rank
    cnt = counts[cell_sorted]
    inA = (wi % 2 == 0) & (wi + 1 < cnt)             # within-cell pair firsts
    pA_in = np.nonzero(inA)[0]
    pB_in = pA_in + 1
    # leftover points (one per odd-count cell), ordered by cell
    lmask = (wi == cnt - 1) & (cnt % 2 == 1)
    lpos = np.nonzero(lmask)[0]
    lcell = cell_sorted[lpos]
    # pair leftovers within runs of consecutive cells
    if len(lpos):
        newrun = np.r_[True, np.diff(lcell) != 1]
        runid = np.cumsum(newrun) - 1
        rstart = np.nonzero(newrun)[0]
        within = np.arange(len(lpos)) - rstart[runid]
        runlen = np.bincount(runid)
        isA = (within % 2 == 0) & (within + 1 < runlen[runid])
        pA_lo = lpos[isA]
        pB_lo = lpos[np.nonzero(isA)[0] + 1]
        solo = (within == runlen[runid] - 1) & (runlen[runid] % 2 == 1)
        pA_solo = lpos[solo]
    else:
        pA_lo = pB_lo = pA_solo = np.zeros(0, np.int64)
    pA = np.concatenate([pA_in, pA_lo, pA_solo])
    pB = np.concatenate([pB_in, pB_lo, np.full(len(pA_solo), -1, np.int64)])
    base = cell_sorted[pA]
    o = np.argsort(base, kind="stable")
    return pA[o], pB[o], base[o]


def _host_prep_weights(w1, b1, w2, b2, w3, b3):
    w1 = np.asarray(w1, np.float32)
    # w1x rows s = h*16 + f*4 + j -> w1[1+f]
    w1x = np.zeros((SLOT, HID), np.float32)
    for h in range(2):
        for f in range(4):
            for j in range(4):
                w1x[h * 16 + f * 4 + j] = w1[1 + f]
    lhsT1 = np.zeros((128, 128), np.float16)
    for o in range(4):
        lhsT1[32 * o:32 * o + 32, 64 * (o % 2):64 * (o % 2) + 64] = w1x
    lhsT12 = np.zeros((2, 128), np.float16)
    lhsT12[0, 0:64] = w1[0]
    lhsT12[1, 64:128] = w1[0]
    lhsT2 = np.zeros((128, 128), np.float16)
    lhsT2[0:64, 0:64] = w2
    lhsT2[64:128, 64:128] = w2
    lhsT3 = np.zeros((128, 8), np.float16)
    lhsT3[0:64, 0:3] = w3
    lhsT3[64:128, 3:6] = w3
    b1rep = np.concatenate([b1, b1]).astype(np.float32).reshape(128, 1)
    b2rep = np.concatenate([b2, b2]).astype(np.float32).reshape(128, 1)
    b3rep = np.zeros((8, 1), np.float32)
    b3rep[0:3, 0] = b3
    b3rep[3:6, 0] = b3
    return lhsT1, lhsT12, lhsT2, lhsT3, b1rep, b2rep, b3rep


def _colmap():
    """Map y3c column Cc in [0, CHPAIRS) -> within-chunk pair rank jc."""
    Cc = np.arange(CHPAIRS)
    qq = Cc // 1024
    rr = Cc % 1024
    half = rr // 512
    bb = (rr % 512) // 128
    c = rr % 128
    return 1024 * qq + 256 * bb + 128 * half + c


def _prep_in_maps(x, emb, w1, b1, w2, b2, w3, b3):
    x = np.asarray(x, np.float32)
    cell, c4, idf = _host_pointdata(x)
    order = np.argsort(cell, kind="stable")
    cs = cell[order]
    pA_s, pB_s, base = _pairing(cs)          # indices into sorted order
    npair_real = len(pA_s)

    n_chunks = -(-npair_real // (N_CORES * CHPAIRS))
    npairs = n_chunks * N_CORES * CHPAIRS
    PAIRS_CORE = n_chunks * CHPAIRS

    # orig-index pair arrays, padded with full dummies
    pA = np.full(npairs, -1, np.int64)
    pB = np.full(npairs, -1, np.int64)
    bases = np.full(npairs, int(base[-1]) if npair_real else 0, np.int64)
    pA[:npair_real] = order[pA_s]
    pB[:npair_real] = np.where(pB_s >= 0, order[np.maximum(pB_s, 0)], -1)
    bases[:npair_real] = base

    # per-point halves: h for B points (0 or 1); A is always h=0
    validB = pB >= 0
    hB = np.zeros(npairs, np.int64)
    hB[validB] = cell[pB[validB]] - bases[validB]
    assert hB.min() >= 0 and hB.max() <= 1

    # c8 per pair half [npairs, 2, 8] fp16
    c8 = np.zeros((npairs, 2, 8), np.float16)
    vA = pA >= 0
    c8[vA, 0, 0:4] = c4[pA[vA]].astype(np.float16)
    c8[validB, 1, 0:4] = np.where((hB[validB] == 0)[:, None],
                                  c4[pB[validB]], 0).astype(np.float16)
    c8[validB, 1, 4:8] = np.where((hB[validB] == 1)[:, None],
                                  c4[pB[validB]], 0).astype(np.float16)
    idfA = np.where(vA, idf[np.maximum(pA, 0)], 0).astype(np.float16)
    idfB = np.where(validB, idf[np.maximum(pB, 0)], 0).astype(np.float16)

    pt16 = _patch_table16(emb)
    ptflat = pt16.reshape(-1)
    win_view = np.lib.stride_tricks.as_strided(
        ptflat, shape=(RX * RY + 1, 128), strides=(32, 2))

    lhsT1, lhsT12, lhsT2, lhsT3, b1rep, b2rep, b3rep = _host_prep_weights(
        w1, b1, w2, b2, w3, b3)
    jcmap = _colmap()

    in_maps = []
    out_pairs = []                            # (origA, origB) per yT column
    for k in range(N_CORES):
        s = k * PAIRS_CORE
        kb = bases[s:s + PAIRS_CORE].reshape(n_chunks, CHPAIRS)
        ptw = np.empty((n_chunks * WIN, 128), np.float16)
        idx16 = np.empty((P, n_chunks * (CHPAIRS // 16)), np.int16)
        for c in range(n_chunks):
            b0 = int(kb[c].min())
            b0 = min(b0, RX * RY + 1 - WIN)
            lo = kb[c] - b0
            assert lo.min() >= 0 and lo.max() < WIN, (
                f"window miss core {k} chunk {c}: {lo.min()} {lo.max()}")
            ptw[c * WIN:(c + 1) * WIN] = win_view[b0:b0 + WIN]
            w16 = lo.astype(np.int16).reshape(CHPAIRS // 16, 16).T
            idx16[:, c * (CHPAIRS // 16):(c + 1) * (CHPAIRS // 16)] = (
                np.tile(w16, (8, 1)))

        # co8: [P, n_chunks*64, 8] : pair jc=(g*128+p) -> co8[p, chunk*64+2g+d]
        kc8 = c8[s:s + PAIRS_CORE].reshape(n_chunks, GCH_, P, 2, 8)
        co8 = np.ascontiguousarray(
            kc8.transpose(2, 0, 1, 3, 4).reshape(P, n_chunks * TCH_, 8))

        # idft: [2, n_chunks*CHPAIRS] in psum column order
        kiA = idfA[s:s + PAIRS_CORE].reshape(n_chunks, CHPAIRS)
        kiB = idfB[s:s + PAIRS_CORE].reshape(n_chunks, CHPAIRS)
        idft = np.empty((2, n_chunks * CHPAIRS), np.float16)
        idft[0] = kiA[:, jcmap].reshape(-1)
        idft[1] = kiB[:, jcmap].reshape(-1)

        # output column -> orig point indices
        kpA = pA[s:s + PAIRS_CORE].reshape(n_chunks, CHPAIRS)
        kpB = pB[s:s + PAIRS_CORE].reshape(n_chunks, CHPAIRS)
        out_pairs.append((kpA[:, jcmap].reshape(-1), kpB[:, jcmap].reshape(-1)))

        in_maps.append({
            "ptw": ptw,
            "idx16": idx16,
            "co8": co8,
            "idft": idft,
            "lhsT1": lhsT1,
            "lhsT12": lhsT12,
            "lhsT2": lhsT2,
            "lhsT3": lhsT3,
            "b1rep": b1rep,
            "b2rep": b2rep,
            "b3rep": b3rep,
        })
    return in_maps, out_pairs, n_chunks


GCH_ = CHPAIRS // P
TCH_ = CHPTS // P

_CACHE = {}


def kernel(x, emb, w1, b1, w2, b2, w3, b3):
    from concourse.bass_utils import run_bass_kernel_spmd

    x = np.asarray(x, np.float32)
    N = x.shape[0]

    in_maps, out_pairs, n_chunks = _prep_in_maps(x, emb, w1, b1, w2, b2, w3, b3)

    key = (n_chunks,)
    if key not in _CACHE:
        nc_new = _build_bass(n_chunks)
        nc_new.compile()
        _CACHE[key] = nc_new
    nc = _CACHE[key]

    trace = os.environ.get("KERNEL_TRACE", "0") == "1"
    res = run_bass_kernel_spmd(
        nc, in_maps, core_ids=list(range(N_CORES)), trace=trace
    )
    if trace and res.exec_time_ns is not None:
        print(f"HW exec time: {res.exec_time_ns} ns")

    y = np.empty((N, 3), np.float32)
    for k in range(N_CORES):
        yT = np.asarray(res.results[k]["yT"], np.float32)
        oA, oB = out_pairs[k]
        mA = oA >= 0
        mB = oB >= 0
        y[oA[mA], :] = yT[0:3, mA].T
        y[oB[mB], :] = yT[3:6, mB].T
    return y
